# revision 1
# baseline (speedup 1.0000x reference)
"""BiDirectional LSTM (B=32, T=512, D=H=512, hard_sigmoid gates, output=fwd+bwd sum)
on 8 Trainium2 NeuronCores.

Sharding: core c in 0..7 -> direction d = c//4 (0=fwd, 1=bwd), batch shard s = c%4
(8 samples each). Backward direction is realized purely in data: the host feeds the
bwd cores time-reversed x; scan outputs stack in iteration order (matching Theano
go_backwards semantics in the reference), so fwd+bwd partial outputs add at equal
step indices.

Per-core program (SPMD, identical on all cores) — all-SBUF, no DRAM scratch:
  Phase 1: xz = 16*(x @ W_cat + b_cat) via PE GEMM (W tiles stationary, xT
           streamed from DRAM in (t,b)-chunks); PSUM results written by ACT
           (bias add + bf16 cast) straight into the resident 128KB/partition
           SBUF xz buffer.
  Phase 2: 512 sequential steps. Each step: 64 128x128 matmul-accumulates
           z16 = xz_t + (16*U_cat).T @ h — i/f/o gate weights in fp8-e4m3
           (FWL loads 4B/cycle, halving the dominant LDWEIGHTS cost; the
           hard_sigmoid saturation absorbs the quantization noise), the
           cell-input c~ gate in bf16. The x16 prescale keeps 16*U in e4m3's
           normal range and is folded into the activation scales (0.2/16,
           1/16) for free. i+f share one PSUM tile and one fused epilogue;
           o's epilogue (the only post-last-matmul tail) runs as back-to-back
           DVE tensor_scalar ops (mult+add, max+min fused pairs) with no
           cross-engine hops. h history (bf16) doubles as the recurrent
           state (no per-step copy) and is bulk-DMA'd out once at the end.
"""

import numpy as np
import ml_dtypes

B, T, D, H = 32, 512, 512, 512
NCORES = 8
BC = B // 4          # 8 samples per core
KT = D // 128        # 4 k-tiles
MT = (4 * H) // 128  # 16 m-tiles (4 gates x 4 chunks)

# fp8-e4m3 recurrent weights for the saturating i/f/o gates (U prescaled x16
# into e4m3's normal range; xz stored as 16*(x@W+b); the 1/16 folds into the
# gate activations' scale). The cell-input c~ gate keeps bf16 weights for
# precision. fp8 FWL loads 4B/cycle -> LDWEIGHTS 27ns vs 53ns per tile.
U_FP8 = True
ZS = 16.0  # pre-activation scale carried by psum/xz


def build(nc, Tn=T, repeat=1):
    import concourse.mybir as mybir
    from concourse.tile import TileContext

    f32 = mybir.dt.float32
    bf16 = mybir.dt.bfloat16
    fp8 = mybir.dt.float8e4
    udt = fp8 if U_FP8 else bf16
    AF = mybir.ActivationFunctionType
    ALU = mybir.AluOpType
    NT = Tn * BC          # GEMM moving free size ((t,b) flattened)
    NCK = min(512, NT)    # phase-1 n-chunk width
    NCH = NT // NCK       # number of n-chunks

    xT = nc.declare_dram_parameter("xT", [KT, 128, NT], bf16, isOutput=False)
    w = nc.declare_dram_parameter("w", [KT, 128, 4 * H], bf16, isOutput=False)
    u8 = nc.declare_dram_parameter("u8", [KT, 128, 3 * H], udt, isOutput=False)
    ub = nc.declare_dram_parameter("ub", [KT, 128, H], bf16, isOutput=False)
    bias = nc.declare_dram_parameter("bias", [128, MT], f32, isOutput=False)
    y = nc.declare_dram_parameter("y", [128, Tn, KT, BC], bf16, isOutput=True)

    with TileContext(nc) as tc:
        with (
            tc.tile_pool(name="const", bufs=1) as cpool,
            tc.tile_pool(name="state", bufs=1) as spool,
        ):
            # Resident across both phases
            u8_sb = [cpool.tile([128, 3 * H], udt, name=f"u8{k}", tag=f"u8{k}") for k in range(KT)]
            ub_sb = [cpool.tile([128, H], bf16, name=f"ub{k}", tag=f"ub{k}") for k in range(KT)]
            w_sb = [cpool.tile([128, 4 * H], bf16, name=f"w{k}", tag=f"w{k}") for k in range(KT)]
            bias_sb = cpool.tile([128, MT], f32, name="bias", tag="bias")
            for k in range(KT):
                nc.sync.dma_start(out=w_sb[k], in_=w[k])
                nc.sync.dma_start(out=u8_sb[k], in_=u8[k])
                nc.sync.dma_start(out=ub_sb[k], in_=ub[k])
            nc.sync.dma_start(out=bias_sb, in_=bias[:])
            half = cpool.tile([128, 1], f32, name="half", tag="half")
            nc.gpsimd.memset(half, 0.5)

            # Gate pre-activations, resident in SBUF: [p, m, (t b)] bf16 (128KB/part)
            xz_sb = spool.tile([128, MT, NT], bf16, name="xz", tag="xz")

            # h history doubles as recurrent state; c state fp32
            y_hist = spool.tile([128, Tn, KT, BC], bf16, name="y_hist", tag="y_hist")
            c_st = spool.tile([128, KT, BC], f32, name="c_st", tag="c_st")
            h0 = spool.tile([128, KT, BC], bf16, name="h0", tag="h0")
            nc.any.memzero(h0)
            nc.any.memzero(c_st)

            # ---------------- Phase 1: input GEMM (xT streamed) ----------------
            with (
                tc.tile_pool(name="gpsum", bufs=2, space="PSUM") as gpsum,
                tc.tile_pool(name="xtp", bufs=2) as xtp,
            ):
                for nci in range(NCH):
                    xt_ch = xtp.tile([128, KT, NCK], bf16, name="xt_ch", tag="xt_ch")
                    for k in range(KT):
                        nc.sync.dma_start(
                            out=xt_ch[:, k], in_=xT[k, :, nci * NCK : (nci + 1) * NCK]
                        )
                    for m in range(MT):
                        ps = gpsum.tile([128, NCK], f32, name="gp", tag="gp")
                        for k in range(KT):
                            nc.tensor.matmul(
                                ps,
                                lhsT=w_sb[k][:, m * 128 : (m + 1) * 128],
                                rhs=xt_ch[:, k],
                                start=(k == 0),
                                stop=(k == KT - 1),
                            )
                        # bias add + bf16 cast straight into resident xz
                        nc.scalar.activation(
                            xz_sb[:, m, nci * NCK : (nci + 1) * NCK],
                            ps,
                            AF.Identity,
                            bias=bias_sb[:, m : m + 1],
                            scale=1.0,
                        )

            # ---------------- Phase 2: recurrence ----------------
            with (
                tc.tile_pool(name="rpsum", bufs=2, space="PSUM") as rpsum,
                tc.tile_pool(name="ztmp", bufs=2) as zpool,
            ):
                for it in range(repeat * Tn):
                    t = it % Tn
                    h_prev = h0 if it == 0 else y_hist[:, (t - 1) % Tn]
                    # U layout gate columns: [i | f | o | c]; emission order
                    # i, f, c~, o -- o last so the c-chain hides under o's
                    # matmuls and the step tail is only o's epilogue.
                    ps_if = rpsum.tile([128, 2 * KT, BC], f32, name="psif", tag="psif")
                    psg = {
                        g: rpsum.tile([128, KT, BC], f32, name=f"ps{g}", tag=f"ps{g}")
                        for g in (3, 2)
                    }
                    # emission: i,f (fused psum), c~, o — o last so the step
                    # tail is only o's epilogue.
                    for m in list(range(8)) + [12, 13, 14, 15, 8, 9, 10, 11]:
                        dst = ps_if[:, m, :] if m < 8 else psg[m // 4][:, m % 4, :]
                        for k in range(KT):
                            lhsT = (
                                ub_sb[k][:, (m - 12) * 128 : (m - 11) * 128]
                                if m >= 12
                                else u8_sb[k][:, m * 128 : (m + 1) * 128]
                            )
                            nc.tensor.matmul(
                                dst,
                                lhsT=lhsT,
                                rhs=h_prev[:, k, :],
                                start=(k == 0),
                                stop=(k == KT - 1),
                            )
                    # i+f gates fused (hard_sigmoid), c~ (tanh) — overlap o's matmuls
                    zif = zpool.tile([128, 2 * KT, BC], f32, name="zif", tag="zif")
                    nc.vector.tensor_add(zif, ps_if, xz_sb[:, 0:8, t * BC : (t + 1) * BC])
                    rif = zpool.tile([128, 2 * KT, BC], f32, name="rif", tag="rif")
                    nc.vector.tensor_scalar(rif, zif, 0.2 / ZS, 0.5, ALU.mult, ALU.add)
                    nc.vector.tensor_scalar(rif, rif, 0.0, 1.0, ALU.max, ALU.min)
                    sig = {0: rif[:, 0:KT], 1: rif[:, KT : 2 * KT]}
                    ztg = zpool.tile([128, KT, BC], f32, name="z3", tag="z3")
                    nc.vector.tensor_add(ztg, psg[3], xz_sb[:, 12:16, t * BC : (t + 1) * BC])
                    gt = zpool.tile([128, KT, BC], f32, name="gt", tag="gt")
                    nc.scalar.activation(gt, ztg, AF.Tanh, scale=1.0 / ZS)
                    # c = f*c + i*g ; tanh(c) — overlaps o's matmuls
                    t1 = zpool.tile([128, KT, BC], f32, name="t1", tag="t1")
                    nc.vector.tensor_mul(t1, sig[1], c_st)
                    t2 = zpool.tile([128, KT, BC], f32, name="t2", tag="t2")
                    nc.vector.tensor_mul(t2, sig[0], gt)
                    nc.vector.tensor_add(c_st, t1, t2)
                    th = zpool.tile([128, KT, BC], f32, name="th", tag="th")
                    nc.scalar.activation(th, c_st, AF.Tanh)
                    # o gate (the only post-last-matmul tail), then h (bf16)
                    zo = zpool.tile([128, KT, BC], f32, name="zo", tag="zo")
                    nc.vector.tensor_add(zo, psg[2], xz_sb[:, 8:12, t * BC : (t + 1) * BC])
                    ro = zpool.tile([128, KT, BC], f32, name="ro", tag="ro")
                    nc.vector.tensor_scalar(ro, zo, 0.2 / ZS, 0.5, ALU.mult, ALU.add)
                    nc.vector.tensor_scalar(ro, ro, 0.0, 1.0, ALU.max, ALU.min)
                    nc.vector.tensor_mul(y_hist[:, t], ro, th)

            nc.sync.dma_start(out=y[:], in_=y_hist)
    return nc


def _prep_core_inputs(x, weights, core, Tn=T):
    """weights: dict with all 24 weight arrays (np float32)."""
    d = core // 4
    s = core % 4
    pre = "" if d == 0 else "b"
    gates = ["i", "f", "o", "c"]
    Wc = np.concatenate([weights[f"W{pre}_{g}"] for g in gates], axis=1)
    Uc = np.concatenate([weights[f"U{pre}_{g}"] for g in gates], axis=1)
    bc = np.concatenate([weights[f"b{pre}_{g}"] for g in gates], axis=0)
    xc = x[s * BC : (s + 1) * BC, :Tn]
    if d == 1:
        xc = xc[:, ::-1]
    # [b, t, d] -> [d, t, b] -> [KT, 128, Tn*BC]
    xTc = np.ascontiguousarray(xc.transpose(2, 1, 0)).reshape(KT, 128, Tn * BC)
    udtype = ml_dtypes.float8_e4m3 if U_FP8 else ml_dtypes.bfloat16
    Us = (ZS * Uc).reshape(KT, 128, 4 * H)
    return {
        "xT": xTc.astype(ml_dtypes.bfloat16),
        "w": (ZS * Wc).reshape(KT, 128, 4 * H).astype(ml_dtypes.bfloat16),
        "u8": np.ascontiguousarray(Us[:, :, : 3 * H]).astype(udtype),
        "ub": np.ascontiguousarray(Us[:, :, 3 * H :]).astype(ml_dtypes.bfloat16),
        "bias": np.ascontiguousarray((ZS * bc).reshape(MT, 128).T).astype(np.float32),
    }


def _gather(results, Tn=T):
    out = np.empty((B, Tn, H), np.float32)
    for s in range(4):
        acc = None
        for d in range(2):
            yc = np.asarray(results[d * 4 + s]["y"], dtype=np.float32)  # [128, Tn, KT, BC]
            part = yc.transpose(3, 1, 2, 0).reshape(BC, Tn, H)
            acc = part if acc is None else acc + part
        out[s * BC : (s + 1) * BC] = acc
    return out


def run(inputs, Tn=T, trace=False):
    import concourse.bacc as bacc
    from concourse.bass_utils import run_bass_kernel_spmd

    x = np.asarray(inputs["x"], np.float32)
    weights = {k: np.asarray(v, np.float32) for k, v in inputs.items() if k != "x"}
    nc = bacc.Bacc("TRN2", target_bir_lowering=False)
    build(nc, Tn)
    nc.compile()
    in_maps = [_prep_core_inputs(x, weights, c, Tn) for c in range(NCORES)]
    res = run_bass_kernel_spmd(nc, in_maps, list(range(NCORES)), trace=trace)
    return _gather(res.results, Tn), res


def kernel(**inputs):
    out, _ = run(inputs)
    return out



# revision 3
# speedup vs baseline: 5.8202x; 5.8202x over previous
"""BiDirectional LSTM (B=32, T=512, D=H=512, hard_sigmoid gates, output=fwd+bwd sum)
on 8 Trainium2 NeuronCores.

Sharding: core c in 0..7 -> direction d = c//4 (0=fwd, 1=bwd), batch shard s = c%4
(8 samples each). Backward direction realized in data: bwd cores get time-reversed
x; scan outputs stack in iteration order (Theano go_backwards semantics), so
fwd+bwd partials add at equal step indices.

The per-core program runs both phases inside hardware For_i loops (dynamic DRAM
offsets via ds()) instead of fully unrolled python loops, keeping the BIR at
~200 instructions -- host-side build/trace, walrus compile and jax lowering
dominate the end-to-end wall clock (HW exec is ~ms), and all of them scale with
instruction count.

  Phase 1 (For_i over (t,b)-chunks): xz = 16*(x @ W_cat + b_cat) via PE GEMM
          (W stationary in SBUF, xT streamed from DRAM), bias+bf16-cast by ACT,
          result staged to a DRAM scratch laid out [128, T, MT, BC].
  Phase 2 (For_i over t): DMA xz_t in (dynamic offset t); 64 128x128
          matmul-accumulates z16 = xz_t + (16*U_cat).T @ h with i/f/o gate
          weights in fp8-e4m3 (halves the dominant LDWEIGHTS cost; the
          hard_sigmoid saturation absorbs the quantization noise) and the
          cell-input c~ gate in bf16. The x16 prescale keeps 16*U in e4m3's
          normal range and folds into the activation scales (0.2/16, 1/16) for
          free. h state lives in a static SBUF tile; the only dynamic APs are
          the two DMAs. h (bf16) is written straight to DRAM y[t] each step.

Execution goes through a local PJRT shard_map runner (same _bass_exec primitive
as bass_utils.run_bass_kernel_spmd's axon path) with two wall-clock tweaks: the
donated output buffers are created on-device (instead of shipping 32MB of host
zeros through the tunnel), and input uploads start before the NEFF compile so
transfer overlaps compilation.
"""

import numpy as np
import ml_dtypes

import jax
import jax.numpy as jnp
from jax.sharding import Mesh, PartitionSpec, NamedSharding

import concourse.bacc as bacc
import concourse.mybir as mybir
from concourse.tile import TileContext
from concourse.bass import ds
from concourse import bass2jax
from concourse.bass2jax import (
    _bass_exec_p,
    partition_id_tensor,
    install_neuronx_cc_hook,
)

from jax.experimental.shard_map import shard_map  # check_rep kwarg API

_DEVICES = jax.devices()  # axon backend init at import time

B, T, D, H = 32, 512, 512, 512
NCORES = 8
BC = B // 4          # 8 samples per core
KT = D // 128        # 4 k-tiles
MT = (4 * H) // 128  # 16 m-tiles (4 gates x 4 chunks)

U_FP8 = True
ZS = 16.0  # pre-activation scale carried by psum/xz


def build(nc, Tn=T):
    f32 = mybir.dt.float32
    bf16 = mybir.dt.bfloat16
    fp8 = mybir.dt.float8e4
    udt = fp8 if U_FP8 else bf16
    AF = mybir.ActivationFunctionType
    ALU = mybir.AluOpType
    NT = Tn * BC          # GEMM moving free size ((t,b) flattened)
    NCK = min(512, NT)    # phase-1 n-chunk width
    NCH = NT // NCK       # number of n-chunks
    TCH = NCK // BC       # t's per chunk

    xT = nc.declare_dram_parameter("xT", [KT, 128, NT], bf16, isOutput=False)
    w = nc.declare_dram_parameter("w", [KT, 128, 4 * H], bf16, isOutput=False)
    u8 = nc.declare_dram_parameter("u8", [KT, 128, 3 * H], udt, isOutput=False)
    ub = nc.declare_dram_parameter("ub", [KT, 128, H], bf16, isOutput=False)
    bias = nc.declare_dram_parameter("bias", [128, MT], f32, isOutput=False)
    y = nc.declare_dram_parameter("y", [128, Tn, KT, BC], bf16, isOutput=True)

    with TileContext(nc) as tc:
        with (
            tc.tile_pool(name="const", bufs=1) as cpool,
            tc.tile_pool(name="state", bufs=1) as spool,
            tc.tile_pool(name="dram", bufs=1, space="DRAM") as dpool,
        ):
            u8_sb = [cpool.tile([128, 3 * H], udt, name=f"u8{k}", tag=f"u8{k}") for k in range(KT)]
            ub_sb = [cpool.tile([128, H], bf16, name=f"ub{k}", tag=f"ub{k}") for k in range(KT)]
            w_sb = [cpool.tile([128, 4 * H], bf16, name=f"w{k}", tag=f"w{k}") for k in range(KT)]
            bias_sb = cpool.tile([128, MT], f32, name="bias", tag="bias")
            for k in range(KT):
                nc.sync.dma_start(out=w_sb[k], in_=w[k])
                nc.sync.dma_start(out=u8_sb[k], in_=u8[k])
                nc.sync.dma_start(out=ub_sb[k], in_=ub[k])
            nc.sync.dma_start(out=bias_sb, in_=bias[:])

            # Gate pre-activations staged in DRAM: [p, t, m, b] bf16
            xz_dram = dpool.tile([128, Tn, MT, BC], bf16, name="xz", tag="xz")
            # Recurrent state (static SBUF addresses)
            h_cur = spool.tile([128, KT, BC], bf16, name="h_cur", tag="h_cur")
            c_st = spool.tile([128, KT, BC], f32, name="c_st", tag="c_st")
            nc.any.memzero(h_cur)
            nc.any.memzero(c_st)

            # ---------------- Phase 1: input GEMM (xT streamed) ----------------
            with (
                tc.tile_pool(name="gpsum", bufs=2, space="PSUM") as gpsum,
                tc.tile_pool(name="xtp", bufs=2) as xtp,
                tc.tile_pool(name="zst", bufs=2) as zst,
            ):
                with tc.For_i(0, NCH, 1) as ci:
                    cflat = ci * NCK
                    ct0 = ci * TCH
                    xt_ch = xtp.tile([128, KT, NCK], bf16, name="xt_ch", tag="xt_ch")
                    for k in range(KT):
                        nc.sync.dma_start(out=xt_ch[:, k], in_=xT[k][:, ds(cflat, NCK)])
                    for m in range(MT):
                        ps = gpsum.tile([128, NCK], f32, name="gp", tag="gp")
                        for k in range(KT):
                            nc.tensor.matmul(
                                ps,
                                lhsT=w_sb[k][:, m * 128 : (m + 1) * 128],
                                rhs=xt_ch[:, k],
                                start=(k == 0),
                                stop=(k == KT - 1),
                            )
                        zm = zst.tile([128, NCK], bf16, name="zm", tag="zm")
                        nc.scalar.activation(zm, ps, AF.Identity, bias=bias_sb[:, m : m + 1], scale=1.0)
                        nc.sync.dma_start(out=xz_dram[:, :, m][:, ds(ct0, TCH)], in_=zm)

            # ---------------- Phase 2: recurrence ----------------
            with (
                tc.tile_pool(name="rpsum", bufs=2, space="PSUM") as rpsum,
                tc.tile_pool(name="ztmp", bufs=2) as zpool,
            ):
                with tc.For_i(0, Tn, 1) as t:
                    zx = zpool.tile([128, MT, BC], bf16, name="zx", tag="zx")
                    nc.sync.dma_start(out=zx, in_=xz_dram[:, ds(t, 1)])
                    # U layout gate columns: [i | f | o | c]; emission order
                    # i, f, c~, o -- o last so the c-chain hides under o's
                    # matmuls and the step tail is only o's epilogue.
                    ps_if = rpsum.tile([128, 2 * KT, BC], f32, name="psif", tag="psif")
                    psg = {
                        g: rpsum.tile([128, KT, BC], f32, name=f"ps{g}", tag=f"ps{g}")
                        for g in (3, 2)
                    }
                    for m in list(range(8)) + [12, 13, 14, 15, 8, 9, 10, 11]:
                        dst = ps_if[:, m, :] if m < 8 else psg[m // 4][:, m % 4, :]
                        for k in range(KT):
                            lhsT = (
                                ub_sb[k][:, (m - 12) * 128 : (m - 11) * 128]
                                if m >= 12
                                else u8_sb[k][:, m * 128 : (m + 1) * 128]
                            )
                            nc.tensor.matmul(
                                dst,
                                lhsT=lhsT,
                                rhs=h_cur[:, k, :],
                                start=(k == 0),
                                stop=(k == KT - 1),
                            )
                    # i+f gates fused (hard_sigmoid), c~ (tanh) overlap o's matmuls
                    zif = zpool.tile([128, 2 * KT, BC], f32, name="zif", tag="zif")
                    nc.vector.tensor_add(zif, ps_if, zx[:, 0:8])
                    rif = zpool.tile([128, 2 * KT, BC], f32, name="rif", tag="rif")
                    nc.vector.tensor_scalar(rif, zif, 0.2 / ZS, 0.5, ALU.mult, ALU.add)
                    nc.vector.tensor_scalar(rif, rif, 0.0, 1.0, ALU.max, ALU.min)
                    ztg = zpool.tile([128, KT, BC], f32, name="z3", tag="z3")
                    nc.vector.tensor_add(ztg, psg[3], zx[:, 12:16])
                    gt = zpool.tile([128, KT, BC], f32, name="gt", tag="gt")
                    nc.scalar.activation(gt, ztg, AF.Tanh, scale=1.0 / ZS)
                    # c = f*c + i*g ; tanh(c)
                    t1 = zpool.tile([128, KT, BC], f32, name="t1", tag="t1")
                    nc.vector.tensor_mul(t1, rif[:, KT : 2 * KT], c_st)
                    t2 = zpool.tile([128, KT, BC], f32, name="t2", tag="t2")
                    nc.vector.tensor_mul(t2, rif[:, 0:KT], gt)
                    nc.vector.tensor_add(c_st, t1, t2)
                    th = zpool.tile([128, KT, BC], f32, name="th", tag="th")
                    nc.scalar.activation(th, c_st, AF.Tanh)
                    # o gate (the only post-last-matmul tail), then h (bf16)
                    zo = zpool.tile([128, KT, BC], f32, name="zo", tag="zo")
                    nc.vector.tensor_add(zo, psg[2], zx[:, 8:12])
                    ro = zpool.tile([128, KT, BC], f32, name="ro", tag="ro")
                    nc.vector.tensor_scalar(ro, zo, 0.2 / ZS, 0.5, ALU.mult, ALU.add)
                    nc.vector.tensor_scalar(ro, ro, 0.0, 1.0, ALU.max, ALU.min)
                    nc.vector.tensor_mul(h_cur, ro, th)
                    nc.sync.dma_start(out=y[:, ds(t, 1)], in_=h_cur)
    return nc


def _prep_dir_weights(weights, d):
    """Per-direction weight prep (shared by the 4 cores of that direction)."""
    pre = "" if d == 0 else "b"
    gates = ["i", "f", "o", "c"]
    Wc = np.concatenate([weights[f"W{pre}_{g}"] for g in gates], axis=1)
    Uc = np.concatenate([weights[f"U{pre}_{g}"] for g in gates], axis=1)
    bc = np.concatenate([weights[f"b{pre}_{g}"] for g in gates], axis=0)
    udtype = ml_dtypes.float8_e4m3 if U_FP8 else ml_dtypes.bfloat16
    Us = (ZS * Uc).reshape(KT, 128, 4 * H)
    return {
        "w": (ZS * Wc).reshape(KT, 128, 4 * H).astype(ml_dtypes.bfloat16),
        "u8": np.ascontiguousarray(Us[:, :, : 3 * H]).astype(udtype),
        "ub": np.ascontiguousarray(Us[:, :, 3 * H :]).astype(ml_dtypes.bfloat16),
        "bias": np.ascontiguousarray((ZS * bc).reshape(MT, 128).T).astype(np.float32),
    }


def _prep_inputs(x, weights, Tn):
    """Build the already-concatenated [8*dim0, ...] global input arrays that
    shard_map slices per-core (avoids one extra copy inside the runner)."""
    bf = ml_dtypes.bfloat16
    x16 = x[:, :Tn].astype(bf)                       # [B, Tn, D]
    # direction-major transpose once: [D, Tn, B]
    xf = np.ascontiguousarray(x16.transpose(2, 1, 0))
    xb = np.ascontiguousarray(xf[:, ::-1, :])
    NT = Tn * BC
    xT_all = np.empty((NCORES * KT, 128, NT), bf)
    for c in range(NCORES):
        src = xf if c < 4 else xb
        s = c % 4
        blk = np.ascontiguousarray(src[:, :, s * BC : (s + 1) * BC])  # [D, Tn, BC]
        xT_all[c * KT : (c + 1) * KT] = blk.reshape(KT, 128, NT)
    wmaps = [_prep_dir_weights(weights, d) for d in range(2)]
    glob = {"xT": xT_all}
    for key in ("w", "u8", "ub", "bias"):
        a0, a1 = wmaps[0][key], wmaps[1][key]
        g = np.empty((NCORES * a0.shape[0], *a0.shape[1:]), a0.dtype)
        n0 = a0.shape[0]
        for c in range(NCORES):
            g[c * n0 : (c + 1) * n0] = a0 if c < 4 else a1
        glob[key] = g
    return glob


def _run_pjrt(nc, glob_inputs, n_cores=NCORES):
    """Execute the prebuilt Bass module via PJRT shard_map (the same
    _bass_exec path as bass_utils.run_bass_kernel_spmd under axon), with
    on-device donated output buffers and upload/compile overlap."""
    install_neuronx_cc_hook()

    partition_name = nc.partition_id_tensor.name if nc.partition_id_tensor else None
    in_names, out_names, out_avals = [], [], []
    for alloc in nc.m.functions[0].allocations:
        if not isinstance(alloc, mybir.MemoryLocationSet):
            continue
        name = alloc.memorylocations[0].name
        if alloc.kind == "ExternalInput":
            if name != partition_name:
                in_names.append(name)
        elif alloc.kind == "ExternalOutput":
            out_names.append(name)
            out_avals.append(
                jax.core.ShapedArray(tuple(alloc.tensor_shape), mybir.dt.np(alloc.dtype))
            )
    if nc.dbg_addr is not None:
        assert not nc.dbg_callbacks
        glob_inputs = dict(glob_inputs)
        glob_inputs[nc.dbg_addr.name] = np.zeros((n_cores, 2), np.uint32)
    n_params = len(in_names)
    n_outs = len(out_avals)
    all_in_names = list(in_names) + list(out_names)
    if partition_name is not None:
        all_in_names.append(partition_name)
    donate = tuple(range(n_params, n_params + n_outs))

    def _body(*args):
        operands = list(args)
        if partition_name is not None:
            operands.append(partition_id_tensor())
        outs = _bass_exec_p.bind(
            *operands,
            out_avals=tuple(out_avals),
            in_names=tuple(all_in_names),
            out_names=tuple(out_names),
            lowering_input_output_aliases=(),
            sim_require_finite=True,
            sim_require_nnan=True,
            nc=nc,
        )
        return tuple(outs)

    devices = _DEVICES[:n_cores]
    mesh = Mesh(np.asarray(devices), ("core",))
    spec = NamedSharding(mesh, PartitionSpec("core"))

    # Kick off input uploads first so the transfer overlaps the NEFF compile.
    dev_in = [jax.device_put(glob_inputs[name], spec) for name in in_names]
    # Donated output buffers created on-device (kernel writes every element of
    # y; the zero fill only serves the donation plumbing).
    zero_shapes = [(n_cores * a.shape[0], *a.shape[1:]) for a in out_avals]
    dev_zeros = jax.jit(
        lambda: tuple(
            jnp.zeros(s, out_avals[i].dtype) for i, s in enumerate(zero_shapes)
        ),
        out_shardings=(spec,) * n_outs,
    )()

    sharded = jax.jit(
        shard_map(
            _body,
            mesh=mesh,
            in_specs=(PartitionSpec("core"),) * (n_params + n_outs),
            out_specs=(PartitionSpec("core"),) * n_outs,
            check_rep=False,
        ),
        donate_argnums=donate,
        keep_unused=True,
    )
    out_arrs = sharded(*dev_in, *dev_zeros)
    return {
        name: np.asarray(out_arrs[i]).reshape(n_cores, *out_avals[i].shape)
        for i, name in enumerate(out_names)
    }


def _gather(y_all, Tn=T):
    """y_all: [8, 128, Tn, KT, BC] bf16 -> [B, Tn, H] f32 (fwd+bwd sum)."""
    out = np.empty((B, Tn, H), np.float32)
    for s in range(4):
        fwd = y_all[s].astype(np.float32)
        bwd = y_all[4 + s].astype(np.float32)
        part = fwd + bwd  # [128, Tn, KT, BC]
        out[s * BC : (s + 1) * BC] = part.transpose(3, 1, 2, 0).reshape(BC, Tn, H)
    return out


def run(inputs, Tn=T, trace=False):
    x = np.asarray(inputs["x"], np.float32)
    weights = {k: np.asarray(v, np.float32) for k, v in inputs.items() if k != "x"}
    nc = bacc.Bacc("TRN2", target_bir_lowering=False)
    build(nc, Tn)
    nc.compile()
    glob = _prep_inputs(x, weights, Tn)
    outs = _run_pjrt(nc, glob)
    res = _Result()
    return _gather(outs["y"], Tn), res


class _Result:
    exec_time_ns = None
    results = None


def kernel(**inputs):
    out, _ = run(inputs)
    return out


# revision 12
# speedup vs baseline: 7.2566x; 1.2468x over previous
"""BiDirectional LSTM (B=32, T=512, D=H=512, hard_sigmoid gates, output=fwd+bwd sum)
on 8 Trainium2 NeuronCores.

Sharding: core c in 0..7 -> direction d = c//4 (0=fwd, 1=bwd), batch shard s = c%4
(8 samples each). Backward direction realized in data: bwd cores get time-reversed
x; scan outputs stack in iteration order (Theano go_backwards semantics), so
fwd+bwd partials add at equal step indices.

The per-core program runs both phases inside hardware For_i loops (dynamic DRAM
offsets via ds()) instead of fully unrolled python loops, keeping the BIR at
~200 instructions -- host-side build/trace, walrus compile and jax lowering
dominate the end-to-end wall clock (HW exec is ~ms), and all of them scale with
instruction count.

  Phase 1 (For_i over (t,b)-chunks): xz = 16*(x @ W_cat + b_cat) via PE GEMM
          (W stationary in SBUF, xT streamed from DRAM), bias+bf16-cast by ACT,
          result staged to a DRAM scratch laid out [128, T, MT, BC].
  Phase 2 (For_i over t): DMA xz_t in (dynamic offset t); 64 128x128
          matmul-accumulates z16 = xz_t + (16*U_cat).T @ h with i/f/o gate
          weights in fp8-e4m3 (halves the dominant LDWEIGHTS cost; the
          hard_sigmoid saturation absorbs the quantization noise) and the
          cell-input c~ gate in bf16. The x16 prescale keeps 16*U in e4m3's
          normal range and folds into the activation scales (0.2/16, 1/16) for
          free. h state lives in a static SBUF tile; the only dynamic APs are
          the two DMAs. h (bf16) is written straight to DRAM y[t] each step.

Execution goes through a local PJRT shard_map runner (same _bass_exec primitive
as bass_utils.run_bass_kernel_spmd's axon path) with two wall-clock tweaks: the
donated output buffers are created on-device (instead of shipping 32MB of host
zeros through the tunnel), and input uploads start before the NEFF compile so
transfer overlaps compilation.
"""

import numpy as np
import ml_dtypes

import jax
import jax.numpy as jnp
from jax.sharding import Mesh, PartitionSpec, NamedSharding

import concourse.bacc as bacc
import concourse.mybir as mybir
from concourse.tile import TileContext
from concourse.bass import ds
from concourse import bass2jax
from concourse.bass2jax import (
    _bass_exec_p,
    partition_id_tensor,
    install_neuronx_cc_hook,
)

from jax.experimental.shard_map import shard_map  # check_rep kwarg API

_DEVICES = jax.devices()  # axon backend init at import time

B, T, D, H = 32, 512, 512, 512
NCORES = 8
BC = B // 4          # 8 samples per core
KT = D // 128        # 4 k-tiles
MT = (4 * H) // 128  # 16 m-tiles (4 gates x 4 chunks)

U_FP8 = True
X_FP8 = False  # fp8 x tested at rel-err 6.8e-2 (fails the 2e-2 gate): the c~
               # tanh path accumulates the quantization noise. Keep x bf16.
ZS = 16.0  # pre-activation scale carried by psum/xz


def build(nc, Tn=T):
    f32 = mybir.dt.float32
    bf16 = mybir.dt.bfloat16
    fp8 = mybir.dt.float8e4
    udt = fp8 if U_FP8 else bf16
    AF = mybir.ActivationFunctionType
    ALU = mybir.AluOpType
    NT = Tn * BC          # GEMM moving free size ((t,b) flattened)
    NCK = min(512, NT)    # phase-1 n-chunk width
    NCH = NT // NCK       # number of n-chunks
    TCH = NCK // BC       # t's per chunk

    xdt = fp8 if X_FP8 else bf16
    xT = nc.declare_dram_parameter("xT", [KT, 128, NT], xdt, isOutput=False)
    w = nc.declare_dram_parameter("w", [KT, 128, 4 * H], bf16, isOutput=False)
    u8 = nc.declare_dram_parameter("u8", [KT, 128, 3 * H], udt, isOutput=False)
    ub = nc.declare_dram_parameter("ub", [KT, 128, H], bf16, isOutput=False)
    bias = nc.declare_dram_parameter("bias", [128, MT], f32, isOutput=False)
    y = nc.declare_dram_parameter("y", [128, Tn, KT, BC], bf16, isOutput=True)

    with TileContext(nc) as tc:
        with (
            tc.tile_pool(name="const", bufs=1) as cpool,
            tc.tile_pool(name="state", bufs=1) as spool,
            tc.tile_pool(name="dram", bufs=1, space="DRAM") as dpool,
        ):
            u8_sb = [cpool.tile([128, 3 * H], udt, name=f"u8{k}", tag=f"u8{k}") for k in range(KT)]
            ub_sb = [cpool.tile([128, H], bf16, name=f"ub{k}", tag=f"ub{k}") for k in range(KT)]
            w_sb = [cpool.tile([128, 4 * H], bf16, name=f"w{k}", tag=f"w{k}") for k in range(KT)]
            bias_sb = cpool.tile([128, MT], f32, name="bias", tag="bias")
            for k in range(KT):
                nc.sync.dma_start(out=w_sb[k], in_=w[k])
                nc.sync.dma_start(out=u8_sb[k], in_=u8[k])
                nc.sync.dma_start(out=ub_sb[k], in_=ub[k])
            nc.sync.dma_start(out=bias_sb, in_=bias[:])

            # Gate pre-activations staged in DRAM: [p, t, m, b] bf16
            xz_dram = dpool.tile([128, Tn, MT, BC], bf16, name="xz", tag="xz")
            # Recurrent state (static SBUF addresses)
            h_cur = spool.tile([128, KT, BC], bf16, name="h_cur", tag="h_cur")
            c_st = spool.tile([128, KT, BC], f32, name="c_st", tag="c_st")
            nc.any.memzero(h_cur)
            nc.any.memzero(c_st)

            # ---------------- Phase 1: input GEMM (xT streamed) ----------------
            with (
                tc.tile_pool(name="gpsum", bufs=2, space="PSUM") as gpsum,
                tc.tile_pool(name="xtp", bufs=2) as xtp,
                tc.tile_pool(name="zst", bufs=2) as zst,
            ):
                with tc.For_i(0, NCH, 1) as ci:
                    cflat = ci * NCK
                    ct0 = ci * TCH
                    xt_ch = xtp.tile([128, KT, NCK], xdt, name="xt_ch", tag="xt_ch")
                    for k in range(KT):
                        nc.sync.dma_start(out=xt_ch[:, k], in_=xT[k][:, ds(cflat, NCK)])
                    for m in range(MT):
                        ps = gpsum.tile([128, NCK], f32, name="gp", tag="gp")
                        for k in range(KT):
                            nc.tensor.matmul(
                                ps,
                                lhsT=w_sb[k][:, m * 128 : (m + 1) * 128],
                                rhs=xt_ch[:, k],
                                start=(k == 0),
                                stop=(k == KT - 1),
                            )
                        zm = zst.tile([128, NCK], bf16, name="zm", tag="zm")
                        nc.scalar.activation(zm, ps, AF.Identity, bias=bias_sb[:, m : m + 1], scale=1.0)
                        nc.sync.dma_start(out=xz_dram[:, :, m][:, ds(ct0, TCH)], in_=zm)

            # ---------------- Phase 2: recurrence ----------------
            with (
                tc.tile_pool(name="rpsum", bufs=2, space="PSUM") as rpsum,
                tc.tile_pool(name="ztmp", bufs=2) as zpool,
            ):
                with tc.For_i(0, Tn, 1) as t:
                    zx = zpool.tile([128, MT, BC], bf16, name="zx", tag="zx")
                    nc.sync.dma_start(out=zx, in_=xz_dram[:, ds(t, 1)])
                    # U layout gate columns: [i | f | o | c]; emission order
                    # i, f, c~, o -- o last so the c-chain hides under o's
                    # matmuls and the step tail is only o's epilogue.
                    ps_if = rpsum.tile([128, 2 * KT, BC], f32, name="psif", tag="psif")
                    psg = {
                        g: rpsum.tile([128, KT, BC], f32, name=f"ps{g}", tag=f"ps{g}")
                        for g in (3, 2)
                    }
                    for m in list(range(8)) + [12, 13, 14, 15, 8, 9, 10, 11]:
                        dst = ps_if[:, m, :] if m < 8 else psg[m // 4][:, m % 4, :]
                        for k in range(KT):
                            lhsT = (
                                ub_sb[k][:, (m - 12) * 128 : (m - 11) * 128]
                                if m >= 12
                                else u8_sb[k][:, m * 128 : (m + 1) * 128]
                            )
                            nc.tensor.matmul(
                                dst,
                                lhsT=lhsT,
                                rhs=h_cur[:, k, :],
                                start=(k == 0),
                                stop=(k == KT - 1),
                            )
                    # i+f gates fused (hard_sigmoid), c~ (tanh) overlap o's matmuls
                    zif = zpool.tile([128, 2 * KT, BC], f32, name="zif", tag="zif")
                    nc.vector.tensor_add(zif, ps_if, zx[:, 0:8])
                    rif = zpool.tile([128, 2 * KT, BC], f32, name="rif", tag="rif")
                    nc.vector.tensor_scalar(rif, zif, 0.2 / ZS, 0.5, ALU.mult, ALU.add)
                    nc.vector.tensor_scalar(rif, rif, 0.0, 1.0, ALU.max, ALU.min)
                    ztg = zpool.tile([128, KT, BC], f32, name="z3", tag="z3")
                    nc.vector.tensor_add(ztg, psg[3], zx[:, 12:16])
                    gt = zpool.tile([128, KT, BC], f32, name="gt", tag="gt")
                    nc.scalar.activation(gt, ztg, AF.Tanh, scale=1.0 / ZS)
                    # c = f*c + i*g ; tanh(c)
                    t1 = zpool.tile([128, KT, BC], f32, name="t1", tag="t1")
                    nc.vector.tensor_mul(t1, rif[:, KT : 2 * KT], c_st)
                    t2 = zpool.tile([128, KT, BC], f32, name="t2", tag="t2")
                    nc.vector.tensor_mul(t2, rif[:, 0:KT], gt)
                    nc.vector.tensor_add(c_st, t1, t2)
                    th = zpool.tile([128, KT, BC], f32, name="th", tag="th")
                    nc.scalar.activation(th, c_st, AF.Tanh)
                    # o gate (the only post-last-matmul tail), then h (bf16)
                    zo = zpool.tile([128, KT, BC], f32, name="zo", tag="zo")
                    nc.vector.tensor_add(zo, psg[2], zx[:, 8:12])
                    ro = zpool.tile([128, KT, BC], f32, name="ro", tag="ro")
                    nc.vector.tensor_scalar(ro, zo, 0.2 / ZS, 0.5, ALU.mult, ALU.add)
                    nc.vector.tensor_scalar(ro, ro, 0.0, 1.0, ALU.max, ALU.min)
                    nc.vector.tensor_mul(h_cur, ro, th)
                    nc.sync.dma_start(out=y[:, ds(t, 1)], in_=h_cur)
    return nc


def _prep_dir_weights(weights, d):
    """Per-direction weight prep (shared by the 4 cores of that direction)."""
    pre = "" if d == 0 else "b"
    gates = ["i", "f", "o", "c"]
    Wc = np.concatenate([weights[f"W{pre}_{g}"] for g in gates], axis=1)
    Uc = np.concatenate([weights[f"U{pre}_{g}"] for g in gates], axis=1)
    bc = np.concatenate([weights[f"b{pre}_{g}"] for g in gates], axis=0)
    udtype = ml_dtypes.float8_e4m3 if U_FP8 else ml_dtypes.bfloat16
    Us = (ZS * Uc).reshape(KT, 128, 4 * H)
    return {
        "w": (ZS * Wc).reshape(KT, 128, 4 * H).astype(ml_dtypes.bfloat16),
        "u8": np.ascontiguousarray(Us[:, :, : 3 * H]).astype(udtype),
        "ub": np.ascontiguousarray(Us[:, :, 3 * H :]).astype(ml_dtypes.bfloat16),
        "bias": np.ascontiguousarray((ZS * bc).reshape(MT, 128).T).astype(np.float32),
    }


def _prep_inputs(x, weights, Tn):
    """Build the already-concatenated [8*dim0, ...] global input arrays that
    shard_map slices per-core (avoids one extra copy inside the runner)."""
    xdt = ml_dtypes.float8_e4m3 if X_FP8 else ml_dtypes.bfloat16
    x16 = x[:, :Tn].astype(xdt)                      # [B, Tn, D]
    # direction-major transpose once: [D, Tn, B]
    xf = np.ascontiguousarray(x16.transpose(2, 1, 0))
    xb = np.ascontiguousarray(xf[:, ::-1, :])
    NT = Tn * BC
    xT_all = np.empty((NCORES * KT, 128, NT), xdt)
    for c in range(NCORES):
        src = xf if c < 4 else xb
        s = c % 4
        blk = np.ascontiguousarray(src[:, :, s * BC : (s + 1) * BC])  # [D, Tn, BC]
        xT_all[c * KT : (c + 1) * KT] = blk.reshape(KT, 128, NT)
    wmaps = [_prep_dir_weights(weights, d) for d in range(2)]
    glob = {"xT": xT_all}
    for key in ("w", "u8", "ub", "bias"):
        a0, a1 = wmaps[0][key], wmaps[1][key]
        g = np.empty((NCORES * a0.shape[0], *a0.shape[1:]), a0.dtype)
        n0 = a0.shape[0]
        for c in range(NCORES):
            g[c * n0 : (c + 1) * n0] = a0 if c < 4 else a1
        glob[key] = g
    return glob


def _run_pjrt(nc, glob_inputs, n_cores=NCORES):
    """Execute the prebuilt Bass module via PJRT shard_map (the same
    _bass_exec path as bass_utils.run_bass_kernel_spmd under axon), with
    on-device donated output buffers and upload/compile overlap."""
    install_neuronx_cc_hook()

    partition_name = nc.partition_id_tensor.name if nc.partition_id_tensor else None
    in_names, out_names, out_avals = [], [], []
    for alloc in nc.m.functions[0].allocations:
        if not isinstance(alloc, mybir.MemoryLocationSet):
            continue
        name = alloc.memorylocations[0].name
        if alloc.kind == "ExternalInput":
            if name != partition_name:
                in_names.append(name)
        elif alloc.kind == "ExternalOutput":
            out_names.append(name)
            out_avals.append(
                jax.core.ShapedArray(tuple(alloc.tensor_shape), mybir.dt.np(alloc.dtype))
            )
    if nc.dbg_addr is not None:
        assert not nc.dbg_callbacks
        glob_inputs = dict(glob_inputs)
        glob_inputs[nc.dbg_addr.name] = np.zeros((n_cores, 2), np.uint32)
    n_params = len(in_names)
    n_outs = len(out_avals)
    all_in_names = list(in_names) + list(out_names)
    if partition_name is not None:
        all_in_names.append(partition_name)
    donate = tuple(range(n_params, n_params + n_outs))

    def _body(*args):
        operands = list(args)
        if partition_name is not None:
            operands.append(partition_id_tensor())
        outs = _bass_exec_p.bind(
            *operands,
            out_avals=tuple(out_avals),
            in_names=tuple(all_in_names),
            out_names=tuple(out_names),
            lowering_input_output_aliases=(),
            sim_require_finite=True,
            sim_require_nnan=True,
            nc=nc,
        )
        return tuple(outs)

    devices = _DEVICES[:n_cores]
    mesh = Mesh(np.asarray(devices), ("core",))
    spec = NamedSharding(mesh, PartitionSpec("core"))

    # Kick off input uploads first so the transfer overlaps the NEFF compile.
    dev_in = [jax.device_put(glob_inputs[name], spec) for name in in_names]
    # Donated output buffers created on-device (kernel writes every element of
    # y; the zero fill only serves the donation plumbing).
    zero_shapes = [(n_cores * a.shape[0], *a.shape[1:]) for a in out_avals]
    dev_zeros = jax.jit(
        lambda: tuple(
            jnp.zeros(s, out_avals[i].dtype) for i, s in enumerate(zero_shapes)
        ),
        out_shardings=(spec,) * n_outs,
    )()

    sharded = jax.jit(
        shard_map(
            _body,
            mesh=mesh,
            in_specs=(PartitionSpec("core"),) * (n_params + n_outs),
            out_specs=(PartitionSpec("core"),) * n_outs,
            check_rep=False,
        ),
        donate_argnums=donate,
        keep_unused=True,
    )
    out_arrs = sharded(*dev_in, *dev_zeros)
    y_glob = out_arrs[0]  # [8*128, Tn, KT, BC] bf16, sharded over cores
    # Fetch the 8 per-core shards concurrently (transfer releases the GIL).
    shards = sorted(
        y_glob.addressable_shards, key=lambda s: s.device.id
    )
    from concurrent.futures import ThreadPoolExecutor

    with ThreadPoolExecutor(max_workers=8) as ex:
        host = list(ex.map(lambda s: np.asarray(s.data), shards))
    return np.stack([h.reshape(*out_avals[0].shape) for h in host])


def _gather(y_all, Tn=T):
    """y_all: [8, 128, Tn, KT, BC] bf16 -> [B, Tn, H] f32 (fwd+bwd sum)."""
    out = np.empty((B, Tn, H), np.float32)
    for s in range(4):
        part = y_all[s].astype(np.float32) + y_all[4 + s].astype(np.float32)
        out[s * BC : (s + 1) * BC] = part.transpose(3, 1, 2, 0).reshape(BC, Tn, H)
    return out


def run(inputs, Tn=T, trace=False):
    x = np.asarray(inputs["x"], np.float32)
    weights = {k: np.asarray(v, np.float32) for k, v in inputs.items() if k != "x"}
    nc = bacc.Bacc("TRN2", target_bir_lowering=False)
    build(nc, Tn)
    nc.compile()
    glob = _prep_inputs(x, weights, Tn)
    y_all = _run_pjrt(nc, glob)
    res = _Result()
    return _gather(y_all, Tn), res


class _Result:
    exec_time_ns = None
    results = None


def kernel(**inputs):
    out, _ = run(inputs)
    return out
